# revision 2
# baseline (speedup 1.0000x reference)
"""Multi-head attention (B=8, S=1024, D=1024, H=16, dh=64) on 8 trn2 cores.

Sharding: data-parallel over batch — one batch element per NeuronCore, no
collectives. Per core the kernel computes, in bf16 with fp32 accumulation:

  Q^T = Wq^T X^T, K^T = Wk^T X^T  (layout [F=head*dh on partitions, S free])
  V   = X Wv                      (layout [S on partitions, F free], with a
                                   ones column appended per head for the
                                   softmax denominator)
  per head h:
    S^T = K_h Q_h^T               ([Sk on partitions, Sq free], K=64 contraction;
                                   even/odd heads sit at partition 0/64 so pairs
                                   run concurrently in separate PE row groups)
    E^T = exp(S^T / 8)            (ScalarE, fused scale, bf16 out)
    [O | d] = E^T.T @ [V_h | 1]   (PSUM [Sq, 65]; col 64 is the denominator)
    out[:, h] = O * (1/d)         (VectorE reciprocal + per-partition scale)

Host side only reshapes: slices the batch, transposes X to X^T and casts
fp32->bf16 (the rounding the on-chip matmuls would apply anyway).
"""

import numpy as np
import ml_dtypes

S = 1024   # sequence length (queries == keys)
D = 1024   # model dim
F = 1024   # heads * head_dim
H = 16
DH = 64
P = 128
NCORES = 8
C = 68     # per-head column stride in the V buffer (64 vals + 1 ones + pad)

_BF16 = ml_dtypes.bfloat16

_cached_nc = None


def _build_nc():
    import concourse.tile as tile
    from concourse import bacc, mybir

    f32 = mybir.dt.float32
    bf16 = mybir.dt.bfloat16
    Exp = mybir.ActivationFunctionType.Exp

    nc = bacc.Bacc("TRN2", target_bir_lowering=False, debug=False,
                   num_devices=NCORES)

    xq_t = nc.dram_tensor("xq_t", [D, S], bf16, kind="ExternalInput").ap()
    xk_t = nc.dram_tensor("xk_t", [D, S], bf16, kind="ExternalInput").ap()
    xv_t = nc.dram_tensor("xv_t", [D, S], bf16, kind="ExternalInput").ap()
    wq = nc.dram_tensor("wq", [D, F], bf16, kind="ExternalInput").ap()
    wk = nc.dram_tensor("wk", [D, F], bf16, kind="ExternalInput").ap()
    wv = nc.dram_tensor("wv", [D, F], bf16, kind="ExternalInput").ap()
    out = nc.dram_tensor("out", [S, F], f32, kind="ExternalOutput").ap()

    KD = D // P   # 8 contraction tiles

    with tile.TileContext(nc) as tc:
        with tc.tile_pool(name="persist", bufs=1) as persist:
            qT = persist.tile([P, KD, S], bf16, tag="qT")
            kT = persist.tile([P, KD, S], bf16, tag="kT")
            v65 = persist.tile([P, S // P, H * C], bf16, tag="v65")
            out_sb = persist.tile([P, S // P, F], f32, tag="out_sb")

            v_heads = v65.rearrange("p s (h c) -> p s h c", c=C)
            nc.vector.memset(v_heads[:, :, :, DH:DH + 1], 1.0)

            # ---- projections: V first (AV needs all of it), then K, Q ----
            with (
                tc.tile_pool(name="proj_in", bufs=2) as proj_in,
                tc.tile_pool(name="proj_ps", bufs=4, space="PSUM") as proj_ps,
            ):
                def load_in(dram):
                    t = proj_in.tile([P, KD, 1024], bf16, tag=dram.name[:1])
                    for dc in range(KD):
                        nc.sync.dma_start(t[:, dc, :],
                                          dram[dc * P:(dc + 1) * P, :])
                    return t

                def proj(lhs_sb, rhs_sb, store):
                    # out[m, n] over 8 m-tiles; contraction over KD k-tiles
                    for m in range(8):
                        pss = [proj_ps.tile([P, 512], f32, tag="pp",
                                            name=f"pp{j}")
                               for j in range(2)]
                        for dc in range(KD):
                            for j in range(2):
                                nc.tensor.matmul(
                                    pss[j][:, :],
                                    lhsT=lhs_sb[:, dc, m * P:(m + 1) * P],
                                    rhs=rhs_sb[:, dc, j * 512:(j + 1) * 512],
                                    start=(dc == 0), stop=(dc == KD - 1),
                                )
                        yield m, pss

                # V = Xv @ Wv : lhsT = Xv^T, rhs = Wv -> [S part, F free]
                xv_sb = load_in(xv_t)
                wv_sb = load_in(wv)
                for m, pss in proj(xv_sb, wv_sb, None):
                    for j in range(2):
                        src = pss[j].rearrange("p (h c) -> p h c", c=DH)
                        dst = v_heads[:, m, j * 8:(j + 1) * 8, 0:DH]
                        nc.vector.tensor_copy(dst, src)

                # K^T = Wk^T Xk^T : lhsT = Wk, rhs = Xk^T -> [F part, S free]
                xk_sb = load_in(xk_t)
                wk_sb = load_in(wk)
                for m, pss in proj(wk_sb, xk_sb, None):
                    for j in range(2):
                        nc.vector.tensor_copy(
                            kT[:, m, j * 512:(j + 1) * 512], pss[j][:, :])

                # Q^T likewise
                xq_sb = load_in(xq_t)
                wq_sb = load_in(wq)
                for m, pss in proj(wq_sb, xq_sb, None):
                    for j in range(2):
                        nc.vector.tensor_copy(
                            qT[:, m, j * 512:(j + 1) * 512], pss[j][:, :])

            # ---- attention, two heads (one partition-pair) at a time ----
            with (
                tc.tile_pool(name="e_pool", bufs=3) as e_pool,
                tc.tile_pool(name="s_ps", bufs=3, space="PSUM") as s_ps,
                tc.tile_pool(name="o_ps", bufs=2, space="PSUM") as o_ps,
                tc.tile_pool(name="r_pool", bufs=4) as r_pool,
            ):
                for hp in range(H // 2):
                    es = [e_pool.tile([P, S // P, S], bf16, tag="e",
                                      name=f"e{i}")
                          for i in range(2)]
                    # scores^T + exp for the head pair, interleaved so the
                    # K=64 matmuls pack into PE row groups 0-1 / 2-3
                    for skm in range(8):
                        pss = [s_ps.tile([P, S], f32, tag="s",
                                          name=f"s{i}")
                               for i in range(2)]
                        for j in range(2):
                            for i in range(2):
                                b0 = i * DH
                                nc.tensor.matmul(
                                    pss[i][:, j * 512:(j + 1) * 512],
                                    lhsT=kT[b0:b0 + DH, hp,
                                            skm * P:(skm + 1) * P],
                                    rhs=qT[b0:b0 + DH, hp,
                                           j * 512:(j + 1) * 512],
                                    start=True, stop=True,
                                )
                        for i in range(2):
                            nc.scalar.activation(es[i][:, skm, :], pss[i][:, :],
                                                 Exp, scale=0.125)
                    # AV + normalize per head
                    for i in range(2):
                        h = 2 * hp + i
                        for sqm in range(8):
                            ps_o = o_ps.tile([P, 512], f32, tag="o")
                            for kt in range(8):
                                nc.tensor.matmul(
                                    ps_o[:, 0:DH + 1],
                                    lhsT=es[i][:, kt, sqm * P:(sqm + 1) * P],
                                    rhs=v65[:, kt, h * C:h * C + DH + 1],
                                    start=(kt == 0), stop=(kt == 7),
                                )
                            rt = r_pool.tile([P, 1], f32, tag="r")
                            nc.vector.reciprocal(rt[:], ps_o[:, DH:DH + 1])
                            nc.vector.tensor_scalar_mul(
                                out_sb[:, sqm, h * DH:(h + 1) * DH],
                                ps_o[:, 0:DH], rt[:])

            for sqm in range(8):
                nc.sync.dma_start(out[sqm * P:(sqm + 1) * P, :],
                                  out_sb[:, sqm, :])

    nc.compile()
    return nc


def _get_nc():
    global _cached_nc
    if _cached_nc is None:
        _cached_nc = _build_nc()
    return _cached_nc


def _in_maps(queries, keys, values, Wq, Wk, Wv):
    wqb = np.ascontiguousarray(Wq).astype(_BF16)
    wkb = np.ascontiguousarray(Wk).astype(_BF16)
    wvb = np.ascontiguousarray(Wv).astype(_BF16)
    maps = []
    for b in range(NCORES):
        maps.append({
            "xq_t": queries[b].T.astype(_BF16),
            "xk_t": keys[b].T.astype(_BF16),
            "xv_t": values[b].T.astype(_BF16),
            "wq": wqb, "wk": wkb, "wv": wvb,
        })
    return maps


def kernel(queries, keys, values, Wq, Wk, Wv, _trace=False):
    from concourse import bass_utils

    nc = _get_nc()
    maps = _in_maps(queries, keys, values, Wq, Wk, Wv)
    res = bass_utils.run_bass_kernel_spmd(
        nc, maps, core_ids=list(range(NCORES)), trace=_trace)
    out = np.stack([res.results[b]["out"] for b in range(NCORES)])
    if _trace:
        kernel.last_results = res
    return out


# revision 3
# speedup vs baseline: 1.0988x; 1.0988x over previous
"""Multi-head attention (B=8, S=1024, D=1024, H=16, dh=64) on 8 trn2 cores.

Sharding: data-parallel over batch — one batch element per NeuronCore, no
collectives. Per core the kernel computes, in fp16 with fp32 accumulation:

  K^T = Wk^T X^T  into a zero-padded [128, H, S] buffer: head h occupies
                  partitions (h%2)*64..+64, the other 64 partitions are zero,
                  so score matmuls contract over K=128 at full PE issue rate.
  Q^T = Wq^T X^T  packed [F on partitions, S free] (head pair per 128)
  V   = X Wv      [S part, F free] with a ones column per head (denominator)
  per head h:
    S^T = K_h Q_h^T        ([Sk part, Sq free]; zero-padded K=128 contraction)
    E^T = exp(S^T / 8)     (ScalarE, fused scale, fp16 out)
    [O | d] = E^T.T [V_h|1]  (PSUM [Sq, 65]; col 64 = softmax denominator)
    out[:, h] = O * (1/d)  (VectorE reciprocal + per-partition scale)

Emission order lets exp (the ScalarE bottleneck) start while the V projection
still owns the PE: K proj, Q proj, scores+exp for head pairs 0-1, V proj,
then the rest of attention.

Host side only reshapes: slices the batch, transposes X to X^T and casts
fp32->fp16 (the rounding the on-chip matmuls would apply anyway).
"""

import numpy as np

S = 1024   # sequence length (queries == keys)
D = 1024   # model dim
F = 1024   # heads * head_dim
H = 16
DH = 64
P = 128
NCORES = 8
C = 68     # per-head column stride in the V buffer (64 vals + 1 ones + pad)

_cached_nc = None


def _build_nc():
    import concourse.tile as tile
    from concourse import bacc, mybir

    f32 = mybir.dt.float32
    f16 = mybir.dt.float16
    Exp = mybir.ActivationFunctionType.Exp

    nc = bacc.Bacc("TRN2", target_bir_lowering=False, debug=False,
                   num_devices=NCORES)

    xq_t = nc.dram_tensor("xq_t", [D, S], f16, kind="ExternalInput").ap()
    xk_t = nc.dram_tensor("xk_t", [D, S], f16, kind="ExternalInput").ap()
    xv_t = nc.dram_tensor("xv_t", [D, S], f16, kind="ExternalInput").ap()
    wq = nc.dram_tensor("wq", [D, F], f16, kind="ExternalInput").ap()
    wk = nc.dram_tensor("wk", [D, F], f16, kind="ExternalInput").ap()
    wv = nc.dram_tensor("wv", [D, F], f16, kind="ExternalInput").ap()
    out = nc.dram_tensor("out", [S, F], f32, kind="ExternalOutput").ap()

    KD = D // P   # 8 contraction tiles

    with tile.TileContext(nc) as tc:
        with tc.tile_pool(name="persist", bufs=1) as persist:
            qT = persist.tile([P, KD, S], f16, tag="qT")
            kTp = persist.tile([P, H, S], f16, tag="kTp")
            v65 = persist.tile([P, S // P, H * C], f16, tag="v65")
            out_sb = persist.tile([P, S // P, F], f32, tag="out_sb")

            v_heads = v65.rearrange("p s (h c) -> p s h c", c=C)
            nc.vector.memset(v_heads[:, :, :, DH:DH + 1], 1.0)
            # zero the unused partition half of each head's K^T slot
            for half in range(2):
                nc.vector.memset(
                    kTp[half * DH:(half + 1) * DH, 1 - half::2, :], 0.0)

            def load_in(pool, dram, tag):
                t = pool.tile([P, KD, 1024], f16, tag=tag)
                for dc in range(KD):
                    nc.sync.dma_start(t[:, dc, :], dram[dc * P:(dc + 1) * P, :])
                return t

            def proj(psum_pool, lhs_sb, rhs_sb):
                for m in range(8):
                    pss = [psum_pool.tile([P, 512], f32, tag="pp",
                                          name=f"pp{j}") for j in range(2)]
                    for dc in range(KD):
                        for j in range(2):
                            nc.tensor.matmul(
                                pss[j][:, :],
                                lhsT=lhs_sb[:, dc, m * P:(m + 1) * P],
                                rhs=rhs_sb[:, dc, j * 512:(j + 1) * 512],
                                start=(dc == 0), stop=(dc == KD - 1),
                            )
                    yield m, pss

            # ---- K^T and Q^T projections ----
            with (
                tc.tile_pool(name="kq_in", bufs=2) as kq_in,
                tc.tile_pool(name="kq_ps", bufs=4, space="PSUM") as kq_ps,
            ):
                xk_sb = load_in(kq_in, xk_t, "x")
                wk_sb = load_in(kq_in, wk, "w")
                for m, pss in proj(kq_ps, wk_sb, xk_sb):
                    # psum rows 0:64 = head 2m, rows 64:128 = head 2m+1
                    for j in range(2):
                        for half in range(2):
                            nc.vector.tensor_copy(
                                kTp[half * DH:(half + 1) * DH, 2 * m + half,
                                    j * 512:(j + 1) * 512],
                                pss[j][half * DH:(half + 1) * DH, :])

                xq_sb = load_in(kq_in, xq_t, "x")
                wq_sb = load_in(kq_in, wq, "w")
                for m, pss in proj(kq_ps, wq_sb, xq_sb):
                    for j in range(2):
                        nc.vector.tensor_copy(
                            qT[:, m, j * 512:(j + 1) * 512], pss[j][:, :])

            # ---- attention (V projection interleaved after 2 head pairs) ----
            with (
                tc.tile_pool(name="e_pool", bufs=3) as e_pool,
                tc.tile_pool(name="s_ps", bufs=2, space="PSUM") as s_ps,
                tc.tile_pool(name="o_ps", bufs=2, space="PSUM") as o_ps,
                tc.tile_pool(name="r_pool", bufs=4) as r_pool,
                tc.tile_pool(name="v_in", bufs=1) as v_in,
                tc.tile_pool(name="v_ps", bufs=2, space="PSUM") as v_ps,
            ):
                def scores_exp(hp):
                    es = [e_pool.tile([P, S // P, S], f16, tag="e",
                                      name=f"e{i}") for i in range(2)]
                    for skm in range(8):
                        for i in range(2):
                            h = 2 * hp + i
                            ps = s_ps.tile([P, S], f32, tag="s")
                            for j in range(2):
                                nc.tensor.matmul(
                                    ps[:, j * 512:(j + 1) * 512],
                                    lhsT=kTp[:, h, skm * P:(skm + 1) * P],
                                    rhs=qT[:, hp, j * 512:(j + 1) * 512],
                                    start=True, stop=True,
                                )
                            nc.scalar.activation(es[i][:, skm, :], ps[:, :],
                                                 Exp, scale=0.125)
                    return es

                def av_norm(hp, es):
                    for i in range(2):
                        h = 2 * hp + i
                        for sqm in range(8):
                            ps_o = o_ps.tile([P, 512], f32, tag="o")
                            for kt in range(8):
                                nc.tensor.matmul(
                                    ps_o[:, 0:DH + 1],
                                    lhsT=es[i][:, kt, sqm * P:(sqm + 1) * P],
                                    rhs=v65[:, kt, h * C:h * C + DH + 1],
                                    start=(kt == 0), stop=(kt == 7),
                                )
                            rt = r_pool.tile([P, 1], f32, tag="r")
                            nc.vector.reciprocal(rt[:], ps_o[:, DH:DH + 1])
                            nc.vector.tensor_scalar_mul(
                                out_sb[:, sqm, h * DH:(h + 1) * DH],
                                ps_o[:, 0:DH], rt[:])

                es01 = [scores_exp(0), scores_exp(1)]

                # V projection: lhsT = Xv^T, rhs = Wv -> [S part, F free]
                xv_sb = load_in(v_in, xv_t, "xv")
                wv_sb = load_in(v_in, wv, "wv")
                for m, pss in proj(v_ps, xv_sb, wv_sb):
                    for j in range(2):
                        src = pss[j].rearrange("p (h c) -> p h c", c=DH)
                        dst = v_heads[:, m, j * 8:(j + 1) * 8, 0:DH]
                        nc.vector.tensor_copy(dst, src)

                av_norm(0, es01[0])
                av_norm(1, es01[1])
                for hp in range(2, H // 2):
                    es = scores_exp(hp)
                    av_norm(hp, es)

            for sqm in range(8):
                nc.sync.dma_start(out[sqm * P:(sqm + 1) * P, :],
                                  out_sb[:, sqm, :])

    nc.compile()
    return nc


def _get_nc():
    global _cached_nc
    if _cached_nc is None:
        _cached_nc = _build_nc()
    return _cached_nc


def _in_maps(queries, keys, values, Wq, Wk, Wv):
    f16 = np.float16
    wqb = np.ascontiguousarray(Wq).astype(f16)
    wkb = np.ascontiguousarray(Wk).astype(f16)
    wvb = np.ascontiguousarray(Wv).astype(f16)
    maps = []
    for b in range(NCORES):
        maps.append({
            "xq_t": queries[b].T.astype(f16),
            "xk_t": keys[b].T.astype(f16),
            "xv_t": values[b].T.astype(f16),
            "wq": wqb, "wk": wkb, "wv": wvb,
        })
    return maps


def kernel(queries, keys, values, Wq, Wk, Wv, _trace=False):
    from concourse import bass_utils

    nc = _get_nc()
    maps = _in_maps(queries, keys, values, Wq, Wk, Wv)
    res = bass_utils.run_bass_kernel_spmd(
        nc, maps, core_ids=list(range(NCORES)), trace=_trace)
    out = np.stack([res.results[b]["out"] for b in range(NCORES)])
    if _trace:
        kernel.last_results = res
    return out


# revision 4
# speedup vs baseline: 1.2257x; 1.1155x over previous
"""Multi-head attention (B=8, S=1024, D=1024, H=16, dh=64) on 8 trn2 cores.

Sharding: data-parallel over batch — one batch element per NeuronCore, no
collectives. Per core the kernel computes, in fp16 with fp32 accumulation:

  K^T = Wk^T X^T  into a zero-padded [128, H, S] buffer: head h occupies
                  partitions (h%2)*64..+64, the other 64 partitions are zero,
                  so score matmuls contract over K=128 at full PE issue rate.
  Q^T = Wq^T X^T  packed [F on partitions, S free] (head pair per 128)
  V   = X Wv      [S part, F free] with a ones column per head (denominator)
  per head h:
    S^T = K_h Q_h^T        ([Sk part, Sq free]; zero-padded K=128 contraction)
    E^T = exp(S^T / 8)     (ScalarE, fused scale, fp16 out)
    [O | d] = E^T.T [V_h|1]  (PSUM [Sq, 65]; col 64 = softmax denominator)
    out[:, h] = O * (1/d)  (VectorE reciprocal + per-partition scale)

Emission order lets exp (the ScalarE bottleneck) start while the V projection
still owns the PE: K proj, Q proj, scores+exp for head pairs 0-1, V proj,
AV pairs 0-1, then the remaining pairs. Output is stored per (pair, row-tile)
so the final DMAs overlap compute instead of queueing in the tail.

Host side only reshapes: slices the batch, transposes X to X^T and casts
fp32->fp16 (the rounding the on-chip matmuls would apply anyway).
"""

import numpy as np

S = 1024   # sequence length (queries == keys)
D = 1024   # model dim
F = 1024   # heads * head_dim
H = 16
DH = 64
P = 128
NCORES = 8
C = 68     # per-head column stride in the V buffer (64 vals + 1 ones + pad)

_cached_nc = None


def _build_nc():
    import concourse.tile as tile
    from concourse import bacc, mybir

    f32 = mybir.dt.float32
    f16 = mybir.dt.float16
    Exp = mybir.ActivationFunctionType.Exp

    nc = bacc.Bacc("TRN2", target_bir_lowering=False, debug=False,
                   num_devices=NCORES)

    xq_t = nc.dram_tensor("xq_t", [D, S], f16, kind="ExternalInput").ap()
    xk_t = nc.dram_tensor("xk_t", [D, S], f16, kind="ExternalInput").ap()
    xv_t = nc.dram_tensor("xv_t", [D, S], f16, kind="ExternalInput").ap()
    wq = nc.dram_tensor("wq", [D, F], f16, kind="ExternalInput").ap()
    wk = nc.dram_tensor("wk", [D, F], f16, kind="ExternalInput").ap()
    wv = nc.dram_tensor("wv", [D, F], f16, kind="ExternalInput").ap()
    out = nc.dram_tensor("out", [S, F], f32, kind="ExternalOutput").ap()

    KD = D // P   # 8 contraction tiles

    with tile.TileContext(nc) as tc:
        with tc.tile_pool(name="persist", bufs=1) as persist:
            qT = persist.tile([P, KD, S], f16, tag="qT")
            kTp = persist.tile([P, H, S], f16, tag="kTp")
            v65 = persist.tile([P, S // P, H * C], f16, tag="v65")

            v_heads = v65.rearrange("p s (h c) -> p s h c", c=C)
            nc.gpsimd.memset(v_heads[:, :, :, DH:DH + 1], 1.0)
            # zero the unused partition half of each head's K^T slot
            for half in range(2):
                nc.gpsimd.memset(
                    kTp[half * DH:(half + 1) * DH, 1 - half::2, :], 0.0)

            def load_pair(pool, dram_x, dram_w, tag):
                tx = pool.tile([P, KD, 1024], f16, tag=tag + "x")
                tw = pool.tile([P, KD, 1024], f16, tag=tag + "w")
                for dc in range(KD):
                    nc.sync.dma_start(tx[:, dc, :],
                                      dram_x[dc * P:(dc + 1) * P, :])
                    nc.sync.dma_start(tw[:, dc, :],
                                      dram_w[dc * P:(dc + 1) * P, :])
                return tx, tw

            def proj(psum_pool, psum_tag, lhs_sb, rhs_sb):
                for m in range(8):
                    pss = [psum_pool.tile([P, 512], f32, tag=psum_tag,
                                          name=f"pj{j}") for j in range(2)]
                    for dc in range(KD):
                        for j in range(2):
                            nc.tensor.matmul(
                                pss[j][:, :],
                                lhsT=lhs_sb[:, dc, m * P:(m + 1) * P],
                                rhs=rhs_sb[:, dc, j * 512:(j + 1) * 512],
                                start=(dc == 0), stop=(dc == KD - 1),
                            )
                    yield m, pss

            # ---- K^T and Q^T projections ----
            with (
                tc.tile_pool(name="kq_in", bufs=2) as kq_in,
                tc.tile_pool(name="kq_ps", bufs=4, space="PSUM") as kq_ps,
            ):
                xk_sb, wk_sb = load_pair(kq_in, xk_t, wk, "i")
                for m, pss in proj(kq_ps, "pp", wk_sb, xk_sb):
                    # psum rows 0:64 = head 2m, rows 64:128 = head 2m+1
                    for j in range(2):
                        for half in range(2):
                            nc.vector.tensor_copy(
                                kTp[half * DH:(half + 1) * DH, 2 * m + half,
                                    j * 512:(j + 1) * 512],
                                pss[j][half * DH:(half + 1) * DH, :])

                xq_sb, wq_sb = load_pair(kq_in, xq_t, wq, "i")
                for m, pss in proj(kq_ps, "pp", wq_sb, xq_sb):
                    for j in range(2):
                        nc.vector.tensor_copy(
                            qT[:, m, j * 512:(j + 1) * 512], pss[j][:, :])

            # ---- attention (V projection interleaved after 2 head pairs) ----
            with (
                tc.tile_pool(name="e_pool", bufs=4) as e_pool,
                tc.tile_pool(name="s_ps", bufs=3, space="PSUM") as s_ps,
                tc.tile_pool(name="o_ps", bufs=2, space="PSUM") as o_ps,
                tc.tile_pool(name="small", bufs=4) as small,
                tc.tile_pool(name="pout", bufs=2) as pout,
                tc.tile_pool(name="v_in", bufs=1) as v_in,
            ):
                def scores_exp(hp):
                    es = [e_pool.tile([P, S // P, S], f16, tag="e",
                                      name=f"e{i}") for i in range(2)]
                    for skm in range(8):
                        for i in range(2):
                            h = 2 * hp + i
                            ps = s_ps.tile([P, S], f32, tag="s")
                            for j in range(2):
                                nc.tensor.matmul(
                                    ps[:, j * 512:(j + 1) * 512],
                                    lhsT=kTp[:, h, skm * P:(skm + 1) * P],
                                    rhs=qT[:, hp, j * 512:(j + 1) * 512],
                                    start=True, stop=True,
                                )
                            nc.scalar.activation(es[i][:, skm, :], ps[:, :],
                                                 Exp, scale=0.125)
                    return es

                def av_norm(hp, es):
                    po = pout.tile([P, S // P, P], f32, tag="po")
                    for sqm in range(8):
                        for i in range(2):
                            h = 2 * hp + i
                            ps_o = o_ps.tile([P, 512], f32, tag="o")
                            for kt in range(8):
                                nc.tensor.matmul(
                                    ps_o[:, 0:DH + 1],
                                    lhsT=es[i][:, kt, sqm * P:(sqm + 1) * P],
                                    rhs=v65[:, kt, h * C:h * C + DH + 1],
                                    start=(kt == 0), stop=(kt == 7),
                                )
                            rt = small.tile([P, 1], f32, tag="r")
                            nc.vector.reciprocal(rt[:], ps_o[:, DH:DH + 1])
                            nc.vector.tensor_scalar_mul(
                                po[:, sqm, i * DH:(i + 1) * DH],
                                ps_o[:, 0:DH], rt[:])
                        nc.sync.dma_start(
                            out[sqm * P:(sqm + 1) * P,
                                hp * P:(hp + 1) * P],
                            po[:, sqm, :])

                es01 = [scores_exp(0), scores_exp(1)]

                # V projection: lhsT = Xv^T, rhs = Wv -> [S part, F free]
                # (psum shared with the AV pool — AV is idle until V exists)
                xv_sb, wv_sb = load_pair(v_in, xv_t, wv, "v")
                for m, pss in proj(o_ps, "o", xv_sb, wv_sb):
                    for j in range(2):
                        src = pss[j].rearrange("p (h c) -> p h c", c=DH)
                        dst = v_heads[:, m, j * 8:(j + 1) * 8, 0:DH]
                        nc.vector.tensor_copy(dst, src)

                av_norm(0, es01[0])
                av_norm(1, es01[1])
                for hp in range(2, H // 2):
                    es = scores_exp(hp)
                    av_norm(hp, es)

    nc.compile()
    return nc


def _get_nc():
    global _cached_nc
    if _cached_nc is None:
        _cached_nc = _build_nc()
    return _cached_nc


def _in_maps(queries, keys, values, Wq, Wk, Wv):
    f16 = np.float16
    wqb = np.ascontiguousarray(Wq).astype(f16)
    wkb = np.ascontiguousarray(Wk).astype(f16)
    wvb = np.ascontiguousarray(Wv).astype(f16)
    maps = []
    for b in range(NCORES):
        maps.append({
            "xq_t": queries[b].T.astype(f16),
            "xk_t": keys[b].T.astype(f16),
            "xv_t": values[b].T.astype(f16),
            "wq": wqb, "wk": wkb, "wv": wvb,
        })
    return maps


def kernel(queries, keys, values, Wq, Wk, Wv, _trace=False):
    from concourse import bass_utils

    nc = _get_nc()
    maps = _in_maps(queries, keys, values, Wq, Wk, Wv)
    res = bass_utils.run_bass_kernel_spmd(
        nc, maps, core_ids=list(range(NCORES)), trace=_trace)
    out = np.stack([res.results[b]["out"] for b in range(NCORES)])
    if _trace:
        kernel.last_results = res
    return out
